# revision 1
# baseline (speedup 1.0000x reference)
import os

os.environ.setdefault("NEURON_CC_FLAGS", "--auto-cast=none")

import numpy as np
import jax
import jax.numpy as jnp
from functools import partial

GROUPS = 8
GP = 64
K = 64
EPS = 1e-5
N_CORES = 8

jax.config.update("jax_default_matmul_precision", "highest")


def _bn_dist(t, g, b, axes, axis_name):
    # training-mode batchnorm with cross-device batch statistics
    m = jax.lax.pmean(t.mean(axes, keepdims=True), axis_name)
    msq = jax.lax.pmean((t * t).mean(axes, keepdims=True), axis_name)
    v = msq - m * m
    shape = [1] * t.ndim
    shape[1] = -1
    return (t - m) * jax.lax.rsqrt(v + EPS) * g.reshape(shape) + b.reshape(shape)


def _fwd_impl(xn, qkv_w, bn_qkv_g, bn_qkv_b, bn_sim_g, bn_sim_b, bn_out_g, bn_out_b,
              weight, gamma, all_emb_q, all_emb_kv):
    # xn: [C, H, W] for this device's batch element n
    C, H, W = xn.shape
    B = W
    xp = xn.transpose(2, 0, 1)                            # [W, C, H] == [B, C, H]
    qkv = jnp.einsum("oc,bch->boh", qkv_w, xp)
    qkv = _bn_dist(qkv, bn_qkv_g, bn_qkv_b, (0, 2), "i")
    qkv = qkv.reshape(B, GROUPS, 2 * GP, H)
    q = qkv[:, :, : GP // 2]
    k = qkv[:, :, GP // 2 : GP]
    v = qkv[:, :, GP:]

    q_emb = all_emb_q[: GP // 2]
    k_emb = all_emb_q[GP // 2 :]
    v_emb = all_emb_kv

    qr = jnp.einsum("bgci,cij->bgij", q, q_emb)
    kr = jnp.einsum("bgci,cij->bgij", k, k_emb).transpose(0, 1, 3, 2)
    qk = jnp.einsum("bgci,bgcj->bgij", q, k)
    stacked = jnp.concatenate([qk, qr, kr], axis=1)
    stacked = _bn_dist(stacked, bn_sim_g, bn_sim_b, (0, 2, 3), "i")
    sim = jax.nn.softmax(stacked.reshape(B, 3, GROUPS, H, H).sum(axis=1), axis=3)

    sv = jnp.matmul(jnp.einsum("bgij,bgcj->bgci", sim, v), weight)
    sve = jnp.matmul(jnp.einsum("bgij,cij->bgci", sim, v_emb), weight)
    out = jnp.concatenate([sv, sve], axis=-1).reshape(B, 2 * GROUPS * GP, H)
    out = _bn_dist(out, bn_out_g, bn_out_b, (0, 2), "i")
    out = out.reshape(W, C, 2, H).sum(axis=2).transpose(1, 2, 0)  # [C, H, W]
    return xn + gamma * out


_fwd = jax.pmap(
    _fwd_impl, axis_name="i",
    in_axes=(0, None, None, None, None, None, None, None, None, None, None, None))

# variant where every arg carries a leading device axis: lets callers pre-stage
# weights on-device once (device_put_replicated) instead of re-broadcasting
_fwd_all0 = jax.pmap(_fwd_impl, axis_name="i")


def kernel(x, qkv_w, bn_qkv_g, bn_qkv_b, bn_sim_g, bn_sim_b, bn_out_g, bn_out_b,
           weight, relative, gamma, pos_map):
    x = np.asarray(x, np.float32)
    # host precompute of the static relative-position gather
    rel_idx = np.arange(K)[:, None] - np.arange(K)[None, :] + K - 1
    all_emb = np.asarray(relative)[:, rel_idx] + np.asarray(pos_map)  # [2*GP, K, K]
    all_emb_q = all_emb[:GP].astype(np.float32)      # q_emb + k_emb halves
    all_emb_kv = all_emb[GP:].astype(np.float32)     # v_emb

    out = _fwd(x,
               np.asarray(qkv_w, np.float32),
               np.asarray(bn_qkv_g, np.float32), np.asarray(bn_qkv_b, np.float32),
               np.asarray(bn_sim_g, np.float32), np.asarray(bn_sim_b, np.float32),
               np.asarray(bn_out_g, np.float32), np.asarray(bn_out_b, np.float32),
               np.asarray(weight, np.float32),
               np.float32(gamma),
               all_emb_q, all_emb_kv)
    return np.asarray(out, np.float32)



# revision 4
# speedup vs baseline: 1.1912x; 1.1912x over previous
import os

os.environ.setdefault("NEURON_CC_FLAGS", "--auto-cast=none")

import numpy as np
import jax
import jax.numpy as jnp

GROUPS = 8
GP = 64
K = 64
EPS = 1e-5
N_CORES = 8

BF = jnp.bfloat16
F32 = jnp.float32


def _fwd_impl(xn, qkv_w, bn_qkv_g, bn_qkv_b, bn_sim_g, bn_sim_b, bn_out_g, bn_out_b,
              weight, gamma, all_emb_q, all_emb_kv):
    # xn: [C, H, W] for this device's batch element n; b == w locally
    C, H, W = xn.shape
    g_, gp = GROUPS, GP

    # ---- qkv 1x1 conv as one dense matmul: [1024, H*W] ----
    xf = xn.reshape(C, H * W)
    qkv = jnp.dot(qkv_w.astype(BF), xf.astype(BF), preferred_element_type=F32)

    # ---- BN1: per-channel batch stats over (b, h) = all cols + devices ----
    s1 = jax.lax.pmean(qkv.mean(1), "i")
    s2 = jax.lax.pmean((qkv * qkv).mean(1), "i")
    sc1 = bn_qkv_g * jax.lax.rsqrt(s2 - s1 * s1 + EPS)
    sh1 = bn_qkv_b - s1 * sc1
    qkv = qkv * sc1[:, None] + sh1[:, None]

    qkv4 = qkv.reshape(g_, 2 * gp, H, W)
    qb = qkv4[:, : gp // 2].astype(BF)       # [g, 32, pos, w]
    kb = qkv4[:, gp // 2: gp].astype(BF)     # [g, 32, pos, w]
    vb = qkv4[:, gp:].astype(BF)             # [g, 64, pos, w]

    q_emb = all_emb_q[: gp // 2].astype(BF)  # [32, i, j]
    k_emb = all_emb_q[gp // 2:].astype(BF)
    v_emb = all_emb_kv.astype(BF)            # [64, i, j]

    # ---- attention logit components (all plain dot_generals) ----
    # reshape every [g, w, i, j] f32 tensor to [(g w), i, j] so the
    # tensorizer sees a 512-deep partition dim (4x128 tiles, 16KB/part free)
    # instead of w=64 partitions x 128KB free, which overflows SBUF.
    qk = jnp.einsum("gciw,gcjw->gwij", qb, kb,
                    preferred_element_type=F32).reshape(g_ * W, H, H)
    qr = jnp.einsum("gciw,cij->gwij", qb, q_emb,
                    preferred_element_type=F32).reshape(g_ * W, H, H)
    # reference: kr = einsum('bgci,cij->bgij', k, k_emb).transpose(..i<->j) —
    # computed directly in transposed orientation (batch over the k position j)
    kr = jnp.einsum("gcjw,cji->gwij", kb, k_emb,
                    preferred_element_type=F32).reshape(g_ * W, H, H)

    # ---- BN2: only the per-(s,g) scale matters — the mean/beta shift is
    # constant along j and cancels in the softmax (exact identity) ----
    def _stats(t):
        tg = t.reshape(g_, W * H * H)
        m = jax.lax.pmean(tg.mean(1), "i")
        ms = jax.lax.pmean((tg * tg).mean(1), "i")
        return m, ms - m * m

    _, v_qk = _stats(qk)
    _, v_qr = _stats(qr)
    _, v_kr = _stats(kr)
    # broadcast per-g scales to [(g w), 1, 1]
    def _bcast(s):
        return jnp.repeat(s, W)[:, None, None]
    s_qk = _bcast(bn_sim_g[0 * g_:1 * g_] * jax.lax.rsqrt(v_qk + EPS))
    s_qr = _bcast(bn_sim_g[1 * g_:2 * g_] * jax.lax.rsqrt(v_qr + EPS))
    s_kr = _bcast(bn_sim_g[2 * g_:3 * g_] * jax.lax.rsqrt(v_kr + EPS))

    sim = jax.nn.softmax(qk * s_qk + qr * s_qr + kr * s_kr,
                         axis=2).astype(BF).reshape(g_, W, H, H)

    # ---- attend + output projection ----
    sv = jnp.einsum("gwij,gcjw->gwci", sim, vb, preferred_element_type=F32)
    sve = jnp.einsum("gwij,cij->gwci", sim, v_emb, preferred_element_type=F32)
    wbf = weight.astype(BF)
    sv2 = jnp.einsum("gwci,ik->gwck", sv.astype(BF), wbf, preferred_element_type=F32)
    sve2 = jnp.einsum("gwci,ik->gwck", sve.astype(BF), wbf, preferred_element_type=F32)

    # ---- BN3 folded with the sv/sve pair-sum: out channel o = g*128 + 2c + p ----
    bn3g = bn_out_g.reshape(g_, gp, 2)
    bn3b = bn_out_b.reshape(g_, gp, 2)

    def _stats2(t):  # t: [g, w, c, h] -> per (g, c) over (w, h) + devices
        m = jax.lax.pmean(t.mean((1, 3)), "i")
        ms = jax.lax.pmean((t * t).mean((1, 3)), "i")
        return m, ms - m * m

    m_a, v_a = _stats2(sv2)
    m_b, v_b = _stats2(sve2)
    a = bn3g[..., 0] * jax.lax.rsqrt(v_a + EPS)      # [g, c]
    b = bn3g[..., 1] * jax.lax.rsqrt(v_b + EPS)
    c3 = (bn3b[..., 0] - m_a * a) + (bn3b[..., 1] - m_b * b)

    # elementwise combine on [(g w), c, h] so partitions tile 4x128
    sv2f = sv2.reshape(g_ * W, gp, H)
    sve2f = sve2.reshape(g_ * W, gp, H)
    af = jnp.repeat(a, W, axis=0).reshape(g_ * W, gp)[:, :, None]
    bf = jnp.repeat(b, W, axis=0).reshape(g_ * W, gp)[:, :, None]
    c3f = jnp.repeat(c3, W, axis=0).reshape(g_ * W, gp)[:, :, None]
    contrib = (sv2f * af + sve2f * bf + c3f).reshape(g_, W, gp, H)
    out = contrib.transpose(0, 2, 3, 1).reshape(C, H, W)
    return xn + gamma * out


_fwd = jax.pmap(
    _fwd_impl, axis_name="i",
    in_axes=(0, None, None, None, None, None, None, None, None, None, None, None))

# variant where every arg carries a leading device axis: lets callers pre-stage
# weights on-device once (device_put_replicated) instead of re-broadcasting
_fwd_all0 = jax.pmap(_fwd_impl, axis_name="i")


def kernel(x, qkv_w, bn_qkv_g, bn_qkv_b, bn_sim_g, bn_sim_b, bn_out_g, bn_out_b,
           weight, relative, gamma, pos_map):
    x = np.asarray(x, np.float32)
    # host precompute of the static relative-position gather
    rel_idx = np.arange(K)[:, None] - np.arange(K)[None, :] + K - 1
    all_emb = np.asarray(relative)[:, rel_idx] + np.asarray(pos_map)  # [2*GP, K, K]
    all_emb_q = all_emb[:GP].astype(np.float32)      # q_emb + k_emb halves
    all_emb_kv = all_emb[GP:].astype(np.float32)     # v_emb

    out = _fwd(x,
               np.asarray(qkv_w, np.float32),
               np.asarray(bn_qkv_g, np.float32), np.asarray(bn_qkv_b, np.float32),
               np.asarray(bn_sim_g, np.float32), np.asarray(bn_sim_b, np.float32),
               np.asarray(bn_out_g, np.float32), np.asarray(bn_out_b, np.float32),
               np.asarray(weight, np.float32),
               np.float32(gamma),
               all_emb_q, all_emb_kv)
    return np.asarray(out, np.float32)


# revision 6
# speedup vs baseline: 13.7717x; 11.5610x over previous
import os

os.environ.setdefault("NEURON_CC_FLAGS", "--auto-cast=none")

import numpy as np
import jax
import jax.numpy as jnp

GROUPS = 8
GP = 64
K = 64
EPS = 1e-5
N_CORES = 8

BF = jnp.bfloat16
F32 = jnp.float32


def _fwd_impl(xn, qkv_w, bn_qkv_g, bn_qkv_b, bn_sim_g, bn_sim_b, bn_out_g, bn_out_b,
              weight, gamma, all_emb_q, all_emb_kv):
    # xn: [C, H, W] for this device's batch element n; b == w locally
    C, H, W = xn.shape
    g_, gp = GROUPS, GP

    # ---- qkv 1x1 conv as one dense matmul: [1024, H*W] ----
    xf = xn.reshape(C, H * W)
    qkv = jnp.dot(qkv_w.astype(BF), xf.astype(BF), preferred_element_type=F32)

    # ---- BN1: per-channel batch stats over (b, h) = all cols + devices ----
    s1 = jax.lax.pmean(qkv.mean(1), "i")
    s2 = jax.lax.pmean((qkv * qkv).mean(1), "i")
    sc1 = bn_qkv_g * jax.lax.rsqrt(s2 - s1 * s1 + EPS)
    sh1 = bn_qkv_b - s1 * sc1
    qkv = qkv * sc1[:, None] + sh1[:, None]

    qkv4 = qkv.reshape(g_, 2 * gp, H, W)
    qb = qkv4[:, : gp // 2].astype(BF)       # [g, 32, pos, w]
    kb = qkv4[:, gp // 2: gp].astype(BF)     # [g, 32, pos, w]
    vb = qkv4[:, gp:].astype(BF)             # [g, 64, pos, w]

    q_emb = all_emb_q[: gp // 2].astype(BF)  # [32, i, j]
    k_emb = all_emb_q[gp // 2:].astype(BF)
    v_emb = all_emb_kv.astype(BF)            # [64, i, j]

    # ---- attention logit components (all plain dot_generals) ----
    # reshape every [g, w, i, j] f32 tensor to [(g w), i, j] so the
    # tensorizer sees a 512-deep partition dim (4x128 tiles, 16KB/part free)
    # instead of w=64 partitions x 128KB free, which overflows SBUF.
    qk = jnp.einsum("gciw,gcjw->gwij", qb, kb,
                    preferred_element_type=F32).reshape(g_ * W, H, H)
    qr = jnp.einsum("gciw,cij->gwij", qb, q_emb,
                    preferred_element_type=F32).reshape(g_ * W, H, H)
    # reference: kr = einsum('bgci,cij->bgij', k, k_emb).transpose(..i<->j) —
    # computed directly in transposed orientation (batch over the k position j)
    kr = jnp.einsum("gcjw,cji->gwij", kb, k_emb,
                    preferred_element_type=F32).reshape(g_ * W, H, H)

    # ---- BN2: only the per-(s,g) scale matters — the mean/beta shift is
    # constant along j and cancels in the softmax (exact identity) ----
    def _stats(t):
        tg = t.reshape(g_, W * H * H)
        m = jax.lax.pmean(tg.mean(1), "i")
        ms = jax.lax.pmean((tg * tg).mean(1), "i")
        return m, ms - m * m

    _, v_qk = _stats(qk)
    _, v_qr = _stats(qr)
    _, v_kr = _stats(kr)
    # broadcast per-g scales to [(g w), 1, 1]
    def _bcast(s):
        return jnp.repeat(s, W)[:, None, None]
    s_qk = _bcast(bn_sim_g[0 * g_:1 * g_] * jax.lax.rsqrt(v_qk + EPS))
    s_qr = _bcast(bn_sim_g[1 * g_:2 * g_] * jax.lax.rsqrt(v_qr + EPS))
    s_kr = _bcast(bn_sim_g[2 * g_:3 * g_] * jax.lax.rsqrt(v_kr + EPS))

    sim = jax.nn.softmax(qk * s_qk + qr * s_qr + kr * s_kr,
                         axis=2).astype(BF).reshape(g_, W, H, H)

    # ---- attend + output projection ----
    sv = jnp.einsum("gwij,gcjw->gwci", sim, vb, preferred_element_type=F32)
    sve = jnp.einsum("gwij,cij->gwci", sim, v_emb, preferred_element_type=F32)
    wbf = weight.astype(BF)
    sv2 = jnp.einsum("gwci,ik->gwck", sv.astype(BF), wbf, preferred_element_type=F32)
    sve2 = jnp.einsum("gwci,ik->gwck", sve.astype(BF), wbf, preferred_element_type=F32)

    # ---- BN3 folded with the sv/sve pair-sum: out channel o = g*128 + 2c + p ----
    bn3g = bn_out_g.reshape(g_, gp, 2)
    bn3b = bn_out_b.reshape(g_, gp, 2)

    def _stats2(t):  # t: [g, w, c, h] -> per (g, c) over (w, h) + devices
        m = jax.lax.pmean(t.mean((1, 3)), "i")
        ms = jax.lax.pmean((t * t).mean((1, 3)), "i")
        return m, ms - m * m

    m_a, v_a = _stats2(sv2)
    m_b, v_b = _stats2(sve2)
    a = bn3g[..., 0] * jax.lax.rsqrt(v_a + EPS)      # [g, c]
    b = bn3g[..., 1] * jax.lax.rsqrt(v_b + EPS)
    c3 = (bn3b[..., 0] - m_a * a) + (bn3b[..., 1] - m_b * b)

    # elementwise combine on [(g w), c, h] so partitions tile 4x128
    sv2f = sv2.reshape(g_ * W, gp, H)
    sve2f = sve2.reshape(g_ * W, gp, H)
    af = jnp.repeat(a, W, axis=0).reshape(g_ * W, gp)[:, :, None]
    bf = jnp.repeat(b, W, axis=0).reshape(g_ * W, gp)[:, :, None]
    c3f = jnp.repeat(c3, W, axis=0).reshape(g_ * W, gp)[:, :, None]
    contrib = (sv2f * af + sve2f * bf + c3f).reshape(g_, W, gp, H)
    out = contrib.transpose(0, 2, 3, 1).reshape(C, H, W)
    return xn + gamma * out


_fwd = jax.pmap(
    _fwd_impl, axis_name="i",
    in_axes=(0, None, None, None, None, None, None, None, None, None, None, None))

# variant where every arg carries a leading device axis: lets callers pre-stage
# weights on-device once (device_put_replicated) instead of re-broadcasting
_fwd_all0 = jax.pmap(_fwd_impl, axis_name="i")


def kernel(x, qkv_w, bn_qkv_g, bn_qkv_b, bn_sim_g, bn_sim_b, bn_out_g, bn_out_b,
           weight, relative, gamma, pos_map):
    x = np.asarray(x, np.float32)
    # host precompute of the static relative-position gather
    rel_idx = np.arange(K)[:, None] - np.arange(K)[None, :] + K - 1
    all_emb = np.asarray(relative)[:, rel_idx] + np.asarray(pos_map)  # [2*GP, K, K]
    all_emb_q = all_emb[:GP].astype(np.float32)      # q_emb + k_emb halves
    all_emb_kv = all_emb[GP:].astype(np.float32)     # v_emb

    out = _fwd(x,
               np.asarray(qkv_w, np.float32),
               np.asarray(bn_qkv_g, np.float32), np.asarray(bn_qkv_b, np.float32),
               np.asarray(bn_sim_g, np.float32), np.asarray(bn_sim_b, np.float32),
               np.asarray(bn_out_g, np.float32), np.asarray(bn_out_b, np.float32),
               np.asarray(weight, np.float32),
               np.float32(gamma),
               all_emb_q, all_emb_kv)
    return np.asarray(out, np.float32)


# revision 9
# speedup vs baseline: 26.9351x; 1.9558x over previous
import os

os.environ.setdefault("NEURON_CC_FLAGS", "--auto-cast=none")

import numpy as np
import jax
import jax.numpy as jnp

GROUPS = 8
GP = 64
K = 64
EPS = 1e-5
N_CORES = 8

BF = jnp.bfloat16
F32 = jnp.float32


def _fwd_impl(xn, qkv_w, bn_qkv_g, bn_qkv_b, bn_sim_g, bn_sim_b, bn_out_g, bn_out_b,
              weight, gamma, all_emb_q, all_emb_kv):
    # xn: [C, H, W] for this device's batch element n; b == w locally
    C, H, W = xn.shape
    g_, gp = GROUPS, GP

    # ---- qkv 1x1 conv as one dense matmul: [1024, H*W] ----
    xf = xn.reshape(C, H * W)
    qkv = jnp.dot(qkv_w.astype(BF), xf.astype(BF), preferred_element_type=F32)

    # ---- BN1: per-channel batch stats over (b, h) = all cols + devices ----
    # (mean and mean-square fused into one allreduce)
    g1 = jax.lax.pmean(jnp.stack([qkv.mean(1), (qkv * qkv).mean(1)]), "i")
    s1, s2 = g1[0], g1[1]
    sc1 = bn_qkv_g * jax.lax.rsqrt(s2 - s1 * s1 + EPS)
    sh1 = bn_qkv_b - s1 * sc1
    qkv = qkv * sc1[:, None] + sh1[:, None]

    qkv4 = qkv.reshape(g_, 2 * gp, H, W)
    qb = qkv4[:, : gp // 2].astype(BF)       # [g, 32, pos, w]
    kb = qkv4[:, gp // 2: gp].astype(BF)     # [g, 32, pos, w]
    vb = qkv4[:, gp:].astype(BF)             # [g, 64, pos, w]

    q_emb = all_emb_q[: gp // 2].astype(BF)  # [32, i, j]
    k_emb = all_emb_q[gp // 2:].astype(BF)
    v_emb = all_emb_kv.astype(BF)            # [64, i, j]

    # ---- attention logit components (all plain dot_generals) ----
    # reshape every [g, w, i, j] f32 tensor to [(g w), i, j] so the
    # tensorizer sees a 512-deep partition dim (4x128 tiles, 16KB/part free)
    # instead of w=64 partitions x 128KB free, which overflows SBUF.
    qk = jnp.einsum("gciw,gcjw->gwij", qb, kb,
                    preferred_element_type=F32).reshape(g_ * W, H, H)
    qr = jnp.einsum("gciw,cij->gwij", qb, q_emb,
                    preferred_element_type=F32).reshape(g_ * W, H, H)
    # reference: kr = einsum('bgci,cij->bgij', k, k_emb).transpose(..i<->j) —
    # computed directly in transposed orientation (batch over the k position j)
    kr = jnp.einsum("gcjw,cji->gwij", kb, k_emb,
                    preferred_element_type=F32).reshape(g_ * W, H, H)

    # ---- BN2: only the per-(s,g) scale matters — the mean/beta shift is
    # constant along j and cancels in the softmax (exact identity).
    # All six stat vectors go through one fused allreduce. ----
    def _locstats(t):
        tg = t.reshape(g_, W * H * H)
        return tg.mean(1), (tg * tg).mean(1)

    loc2 = jnp.stack([*_locstats(qk), *_locstats(qr), *_locstats(kr)])  # [6, g]
    gl2 = jax.lax.pmean(loc2, "i")

    # broadcast per-g scales to [(g w), 1, 1]
    def _bcast(s):
        return jnp.repeat(s, W)[:, None, None]

    def _scale(idx, gvec):
        v = gl2[2 * idx + 1] - gl2[2 * idx] * gl2[2 * idx]
        return _bcast(gvec * jax.lax.rsqrt(v + EPS))

    s_qk = _scale(0, bn_sim_g[0 * g_:1 * g_])
    s_qr = _scale(1, bn_sim_g[1 * g_:2 * g_])
    s_kr = _scale(2, bn_sim_g[2 * g_:3 * g_])

    sim = jax.nn.softmax(qk * s_qk + qr * s_qr + kr * s_kr,
                         axis=2).astype(BF).reshape(g_, W, H, H)

    # ---- attend + output projection ----
    sv = jnp.einsum("gwij,gcjw->gwci", sim, vb, preferred_element_type=F32)
    sve = jnp.einsum("gwij,cij->gwci", sim, v_emb, preferred_element_type=F32)
    wbf = weight.astype(BF)
    sv2 = jnp.einsum("gwci,ik->gwck", sv.astype(BF), wbf, preferred_element_type=F32)
    sve2 = jnp.einsum("gwci,ik->gwck", sve.astype(BF), wbf, preferred_element_type=F32)

    # ---- BN3 folded with the sv/sve pair-sum: out channel o = g*128 + 2c + p ----
    bn3g = bn_out_g.reshape(g_, gp, 2)
    bn3b = bn_out_b.reshape(g_, gp, 2)

    def _locstats2(t):  # t: [g, w, c, h] -> per (g, c) over (w, h)
        return t.mean((1, 3)), (t * t).mean((1, 3))

    loc3 = jnp.stack([*_locstats2(sv2), *_locstats2(sve2)])  # [4, g, c]
    gl3 = jax.lax.pmean(loc3, "i")
    m_a, v_a = gl3[0], gl3[1] - gl3[0] * gl3[0]
    m_b, v_b = gl3[2], gl3[3] - gl3[2] * gl3[2]
    a = bn3g[..., 0] * jax.lax.rsqrt(v_a + EPS)      # [g, c]
    b = bn3g[..., 1] * jax.lax.rsqrt(v_b + EPS)
    c3 = (bn3b[..., 0] - m_a * a) + (bn3b[..., 1] - m_b * b)

    # elementwise combine on [(g w), c, h] so partitions tile 4x128
    sv2f = sv2.reshape(g_ * W, gp, H)
    sve2f = sve2.reshape(g_ * W, gp, H)
    af = jnp.repeat(a, W, axis=0).reshape(g_ * W, gp)[:, :, None]
    bf = jnp.repeat(b, W, axis=0).reshape(g_ * W, gp)[:, :, None]
    c3f = jnp.repeat(c3, W, axis=0).reshape(g_ * W, gp)[:, :, None]
    contrib = (sv2f * af + sve2f * bf + c3f).reshape(g_, W, gp, H)
    out = contrib.transpose(0, 2, 3, 1).reshape(C, H, W)
    return xn + gamma * out


_fwd = jax.pmap(
    _fwd_impl, axis_name="i",
    in_axes=(0, None, None, None, None, None, None, None, None, None, None, None))

# variant where every arg carries a leading device axis: lets callers pre-stage
# weights on-device once (device_put_replicated) instead of re-broadcasting
_fwd_all0 = jax.pmap(_fwd_impl, axis_name="i")


def kernel(x, qkv_w, bn_qkv_g, bn_qkv_b, bn_sim_g, bn_sim_b, bn_out_g, bn_out_b,
           weight, relative, gamma, pos_map):
    x = np.asarray(x, np.float32)
    # host precompute of the static relative-position gather
    rel_idx = np.arange(K)[:, None] - np.arange(K)[None, :] + K - 1
    all_emb = np.asarray(relative)[:, rel_idx] + np.asarray(pos_map)  # [2*GP, K, K]
    all_emb_q = all_emb[:GP].astype(np.float32)      # q_emb + k_emb halves
    all_emb_kv = all_emb[GP:].astype(np.float32)     # v_emb

    out = _fwd(x,
               np.asarray(qkv_w, np.float32),
               np.asarray(bn_qkv_g, np.float32), np.asarray(bn_qkv_b, np.float32),
               np.asarray(bn_sim_g, np.float32), np.asarray(bn_sim_b, np.float32),
               np.asarray(bn_out_g, np.float32), np.asarray(bn_out_b, np.float32),
               np.asarray(weight, np.float32),
               np.float32(gamma),
               all_emb_q, all_emb_kv)
    return np.asarray(out, np.float32)
